# revision 46
# baseline (speedup 1.0000x reference)
"""Self-contained Trainium2 Bass kernel for nn_Attention (8-head self-attention).

Reference computation (per batch element b):
    xt = x[b].reshape(C, N).T            # (N, C),  N = H*W = 1024
    q  = xt @ Wq                         # (N, 512)
    k, v = split(xt @ Wkv)               # (N, 512) each
    per head h (d=64): sim = q_h k_h^T / 8 ; P = softmax(sim) ; o_h = P v_h
    out[b] = concat_h(o_h) @ Wo + bo     # (N, C)

Sharding: pure data parallel -- core b computes batch element b (8 cores, 8
batch elements, no collectives).

Layout strategy (keeps every matmul contraction dim on partitions, with zero
on-chip transposes):
  - x[b] is used as (C, N): already the transpose of xt.
  - qT, kT are produced in (inner, N) layout; V in (N, inner) layout with an
    extra ones-column per head so the P@V matmul also emits the softmax
    denominators (M = 64+1 = 65).
  - simT tiles are (key j on partitions, query i on free dim); exp runs on
    ScalarE straight out of PSUM into bf16 SBUF.
  - O'^T / denominators are normalized via DVE reciprocal + gpsimd
    partition-broadcast, then the output projection runs on the normalized
    bf16 O^T.
"""

import numpy as np

import concourse.bass as bass
import concourse.mybir as mybir
import concourse.tile as tile
from concourse import bacc

B, C, N = 8, 512, 1024
HEADS, D = 8, 64
INNER = HEADS * D  # 512
SCALE = D ** -0.5
P = 128
CT = C // P       # 4  k-tiles over C
MT = INNER // P   # 4  partition-tiles over inner
JT = N // P       # 8  key tiles
NT = N // P       # 8  output row tiles
NB = N // 512     # 2  free-dim blocks of 512 over N

F32 = mybir.dt.float32
BF16 = mybir.dt.bfloat16
EXP = mybir.ActivationFunctionType.Exp


def build_nc(debug=False):
    nc = bacc.Bacc(
        "TRN2", target_bir_lowering=False, debug=debug, num_devices=B
    )
    x_d = nc.dram_tensor("x", [C, N], F32, kind="ExternalInput")
    wq_d = nc.dram_tensor("Wq", [C, INNER], F32, kind="ExternalInput")
    wkv_d = nc.dram_tensor("Wkv", [C, 2 * INNER], F32, kind="ExternalInput")
    wo_d = nc.dram_tensor("Wo", [INNER, C], F32, kind="ExternalInput")
    bo_d = nc.dram_tensor("bo", [C], F32, kind="ExternalInput")
    out_d = nc.dram_tensor("out", [N, C], F32, kind="ExternalOutput")

    with tile.TileContext(nc) as tc:
        with (
            tc.tile_pool(name="persist", bufs=1) as persist,
            tc.tile_pool(name="stage", bufs=1) as stage,
            tc.tile_pool(name="etp", bufs=2) as etp,
            tc.tile_pool(name="ovp", bufs=4) as ovp,
            tc.tile_pool(name="small", bufs=2) as small,
            tc.tile_pool(name="outs", bufs=3) as outs,
            tc.tile_pool(name="dramp", bufs=2, space="DRAM") as dramp,
            tc.tile_pool(name="psA", bufs=2, space="PSUM") as psA,
            tc.tile_pool(name="psS", bufs=1, space="PSUM") as psS,
            tc.tile_pool(name="psO", bufs=1, space="PSUM") as psO,
        ):
            # ---------------- load + cast inputs ----------------
            # x and Wq are loaded/cast per 128-row chunk so the first q-proj
            # matmuls can start as soon as chunk 0 lands.
            x_f = stage.tile([P, CT, N], F32, tag="st_x")
            x_b = persist.tile([P, CT, N], BF16)
            x_dv = x_d[:].rearrange("(a p) n -> p a n", p=P)
            wq_f = stage.tile([P, CT, INNER], F32, tag="st_q")
            wq_b = persist.tile([P, CT, INNER], BF16)
            wq_dv = wq_d[:].rearrange("(a p) m -> p a m", p=P)
            for a in range(CT):
                nc.sync.dma_start(out=x_f[:, a, :], in_=x_dv[:, a, :])
                nc.vector.tensor_copy(out=x_b[:, a, :], in_=x_f[:, a, :])
                nc.sync.dma_start(out=wq_f[:, a, :], in_=wq_dv[:, a, :])
                nc.vector.tensor_copy(out=wq_b[:, a, :], in_=wq_f[:, a, :])

            wkv_f = stage.tile([P, CT, 2 * INNER], F32, tag="st_x")
            nc.sync.dma_start(out=wkv_f, in_=wkv_d[:].rearrange("(a p) m -> p a m", p=P))
            wkv_b = persist.tile([P, CT, 2 * INNER], BF16)
            nc.vector.tensor_copy(out=wkv_b, in_=wkv_f)

            wo_f = stage.tile([P, MT, C], F32, tag="st_q")
            nc.sync.dma_start(out=wo_f, in_=wo_d[:].rearrange("(a p) m -> p a m", p=P))
            wo_b = persist.tile([P, MT, C], BF16)
            nc.vector.tensor_copy(out=wo_b, in_=wo_f)

            bo_bc = persist.tile([P, C], F32)
            bo_ap = bo_d[:]
            nc.gpsimd.dma_start(
                out=bo_bc,
                in_=bass.AP(tensor=bo_ap.tensor, offset=bo_ap.offset,
                            ap=[[0, P], [1, C]]),
            )

            zb = persist.tile([P, 1], F32)
            nc.vector.memset(zb, 0.0)

            # ---------------- projections ----------------
            # qT, kT: (inner, N) transposed layout; inner = mt*128 + p
            qT = persist.tile([P, MT, N], BF16)
            kT = persist.tile([P, MT, N], BF16)
            for dst, col0 in ((qT, None), (kT, 0)):
                w_b = wq_b if col0 is None else wkv_b
                base = 0 if col0 is None else col0
                for mt in range(MT):
                    for ib in range(NB):
                        ps = psA.tile([P, 512], F32, tag="proj")
                        for a in range(CT):
                            nc.tensor.matmul(
                                ps,
                                lhsT=w_b[:, a, base + mt * P: base + (mt + 1) * P],
                                rhs=x_b[:, a, ib * 512:(ib + 1) * 512],
                                start=(a == 0),
                                stop=(a == CT - 1),
                            )
                        nc.vector.tensor_copy(
                            out=dst[:, mt, ib * 512:(ib + 1) * 512], in_=ps)

            # V in normal layout (token j on partitions), per head with an
            # extra ones column: v_ext[:, jt, h, 0:64] = V, [..., 64] = 1
            v_ext = persist.tile([P, JT, HEADS, D + 1], BF16)
            nc.vector.memset(v_ext[:, :, :, D], 1.0)
            for jt in range(JT):
                ps = psA.tile([P, 512], F32, tag="proj")
                for a in range(CT):
                    nc.tensor.matmul(
                        ps,
                        lhsT=x_b[:, a, jt * P:(jt + 1) * P],
                        rhs=wkv_b[:, a, INNER:2 * INNER],
                        start=(a == 0),
                        stop=(a == CT - 1),
                    )
                nc.vector.tensor_copy(
                    out=v_ext[:, jt, :, 0:D],
                    in_=ps[:].rearrange("p (h d) -> p h d", h=HEADS),
                )

            # ---------------- attention (head pairs) ----------------
            # Heads 2p/2p+1 sit at partition bases 0/64 of qT/kT tile mt=p;
            # their K=64 sim matmuls land on different PE row-groups, so
            # adjacent issue -> concurrent execution + LDWEIGHTS pull-ahead.
            oT = persist.tile([P, MT, N], BF16)  # normalized O^T, packed like qT
            pending_finish = None
            for h in range(HEADS):
                hp = (h % 2) * D
                hm = h // 2
                if h % 2 == 0:
                    # sim + exp for BOTH heads of the pair, interleaved
                    qA = qT[0:D, hm, :]
                    kA = kT[0:D, hm, :]
                    qB = qT[D:2 * D, hm, :]
                    kB = kT[D:2 * D, hm, :]
                    etA = etp.tile([P, JT, N], BF16, tag="etA")
                    etB = etp.tile([P, JT, N], BF16, tag="etB")
                    for jt in range(JT):
                        stA = psS.tile([P, N], F32, tag="stA")
                        stB = psS.tile([P, N], F32, tag="stB")
                        for ib in range(NB):
                            nc.tensor.matmul(
                                stA[:, ib * 512:(ib + 1) * 512],
                                lhsT=kA[:, jt * P:(jt + 1) * P],
                                rhs=qA[:, ib * 512:(ib + 1) * 512],
                                start=True,
                                stop=True,
                            )
                            nc.tensor.matmul(
                                stB[:, ib * 512:(ib + 1) * 512],
                                lhsT=kB[:, jt * P:(jt + 1) * P],
                                rhs=qB[:, ib * 512:(ib + 1) * 512],
                                start=True,
                                stop=True,
                            )
                        nc.scalar.activation(
                            out=etA[:, jt, :], in_=stA, func=EXP,
                            bias=zb, scale=SCALE)
                        nc.scalar.activation(
                            out=etB[:, jt, :], in_=stB, func=EXP,
                            bias=zb, scale=SCALE)
                    et = etA
                else:
                    et = etB

                # O'^T_ext = [V_h | 1]^T @ E^T ; row D is the softmax denom
                ov = psO.tile([D + 1, N], F32, tag="ov")
                for jt in range(JT):
                    for ib in range(NB):
                        nc.tensor.matmul(
                            ov[:, ib * 512:(ib + 1) * 512],
                            lhsT=v_ext[:, jt, h, :],
                            rhs=et[:, jt, ib * 512:(ib + 1) * 512],
                            start=(jt == 0),
                            stop=(jt == JT - 1),
                        )
                # two quick copies release the PSUM tile for the next head;
                # the s-row copy goes first since it gates the recip chain
                s_tmp = small.tile([1, N], F32, tag="stmp")
                nc.vector.tensor_copy(out=s_tmp, in_=ov[D:D + 1, :])
                ov_sb = ovp.tile([D, N], BF16, tag="ovsb")
                nc.vector.tensor_copy(out=ov_sb, in_=ov[0:D, :])

                # 1/denom, computed 128 lanes wide: bounce the row through
                # DRAM to spread it across partitions (DRAM APs have no
                # partition-base restrictions). DMAs ride the otherwise-idle
                # gpsimd queue; the recip+mul are deferred one head so their
                # DMA-chain latency never head-of-line-blocks the DVE FIFO.
                sd = dramp.tile([N], F32, tag="sd")
                nc.sync.dma_start(out=sd, in_=s_tmp)
                # contiguous 8-elem chunks per partition: 32B descriptors
                # (a strided per-element spread would be descriptor-bound)
                st2 = small.tile([P, NT], F32, tag="st2")
                nc.sync.dma_start(
                    out=st2, in_=sd.rearrange("(p k) -> p k", k=NT))

                def finish(h=h, ov_sb=ov_sb, st2=st2):
                    rst2 = small.tile([P, NT], F32, tag="rst2")
                    nc.vector.reciprocal(rst2, st2)
                    rsd = dramp.tile([N], F32, tag="rsd")
                    nc.sync.dma_start(
                        out=rsd.rearrange("(p k) -> p k", k=NT), in_=rst2)
                    rep = small.tile([D, N], F32, tag="rep")
                    rsd_ap = rsd[:]
                    nc.sync.dma_start(
                        out=rep,
                        in_=bass.AP(tensor=rsd_ap.tensor, offset=rsd_ap.offset,
                                    ap=[[0, D], [1, N]]),
                    )
                    hp = (h % 2) * D
                    nc.vector.tensor_mul(oT[hp:hp + D, h // 2, :], ov_sb, rep)

                if pending_finish is not None:
                    pending_finish()
                pending_finish = finish
            pending_finish()

            # ---------------- output projection ----------------
            for it in range(NT):
                pf = psA.tile([P, C], F32, tag="proj")
                for kk in range(MT):
                    nc.tensor.matmul(
                        pf,
                        lhsT=oT[:, kk, it * P:(it + 1) * P],
                        rhs=wo_b[:, kk, :],
                        start=(kk == 0),
                        stop=(kk == MT - 1),
                    )
                fin = outs.tile([P, C], F32, tag="fin")
                nc.vector.tensor_add(fin, pf, bo_bc)
                nc.sync.dma_start(out=out_d[it * P:(it + 1) * P, :], in_=fin)

    return nc


def kernel(x, Wq, Wkv, Wo, bo):
    from concourse.bass_utils import run_bass_kernel_spmd

    nc = build_nc()
    nc.compile()
    xs = np.ascontiguousarray(x.reshape(B, C, N)).astype(np.float32, copy=False)
    in_maps = [
        {
            "x": xs[b],
            "Wq": np.asarray(Wq, dtype=np.float32),
            "Wkv": np.asarray(Wkv, dtype=np.float32),
            "Wo": np.asarray(Wo, dtype=np.float32),
            "bo": np.asarray(bo, dtype=np.float32),
        }
        for b in range(B)
    ]
    res = run_bass_kernel_spmd(nc, in_maps, list(range(B)))
    return np.stack([res.results[b]["out"] for b in range(B)], axis=0)
